# revision 6
# baseline (speedup 1.0000x reference)
"""Trainium2 Bass kernel for nn_NeuralODESolver (Tsit5 neural-ODE integrator).

Strategy (data-parallel across 8 NeuronCores, 2-way interleaved per core):
  - Shard the batch dim (1024) into 8 x 128; each core further splits its
    batch into two independent 64-wide halves (A/B) whose Tsit5 chains are
    interleaved so the tensor engine always has ready work. This both hides
    the per-stage dependency-chain latency and keeps the PE busy enough for
    the HAM clock gate to hold the 2.4 GHz warm state (the previous
    single-chain kernel ran the whole integration at the cold 1.2 GHz clock).
  - Plain Tsit5 stage structure (no layer-3-into-layer-1 fusion): 8-10
    matmuls per stage-half at N=64, which the warm PE sustains at ~34ns each.
  - Bias/forcing folding: z tiles are [y; I64] (or [I64; y]) in fp16; the L1
    stationary is [W1y^T ; (W1u@u + b1)^T] so the constant forcing term and
    b1 enter through the identity carrier — u never ships to the device and
    relu1 needs no bias. b2 is prefilled into PSUM with one identity matmul
    per stage, so both relus are single zero-bias ACT ops over the merged
    [128, 2x64] PSUM tile.
  - L3 computes duplicated [k; k] (stationary [W3^T|W3^T]) so every
    Runge-Kutta scatter op covers two fp32 accumulator targets per
    instruction. Accumulator pairs: P34=[zb3;zb4], P56=[zb5;zb6],
    ynew=[y';y'], with z4/z6 stored flipped ([I64; y]) to keep every
    scatter lane-aligned.
  - Engine balance per stage-half: PE 9-10 MMs; ACT merged relu1+relu2;
    DVE z-write STT (PSUM) + k->SBUF copy; GpSimd (otherwise idle) runs the
    fp32 accumulator updates from the SBUF copy of k (GPSIMD cannot touch
    PSUM). ~18 warmup matmuls at kernel start span the HAM activity window
    while the weight DMAs land.
"""

import numpy as np

# Tsitouras 5(4) tableau (5th-order weights; b7 = 0)
_A21 = 0.161
_A31, _A32 = -0.008480655492356989, 0.335480655492357
_A41, _A42, _A43 = 2.8971530571054935, -6.359448489975075, 4.3622954328695815
_A51, _A52, _A53, _A54 = 5.325864828439257, -11.748883564062828, 7.4955393428898365, -0.09249506636175525
_A61, _A62, _A63, _A64, _A65 = 5.86145544294642, -12.92096931784711, 8.159367898576159, -0.071584973281401, -0.028269050394068383
_B1, _B2, _B3, _B4, _B5, _B6 = 0.09646076681806523, 0.01, 0.4798896504144996, 1.379008574103742, -3.290069515436081, 2.324710524099774

SECOND = 1.0 / 3600.0
DT0 = 60.0

N_CORES = 8
NH = 64  # half-batch width per core

N_WARMUP_MM = 18


def _build_program(n_steps, b3_nonzero):
    import concourse.mybir as mybir
    import concourse.tile as tile
    from concourse.tile import add_dep_helper
    from concourse import bacc

    f16 = mybir.dt.float16
    f32 = mybir.dt.float32
    Relu = mybir.ActivationFunctionType.Relu
    MUL = mybir.AluOpType.mult
    ADD = mybir.AluOpType.add
    MAX = mybir.AluOpType.max

    h = DT0 * SECOND
    hA = {
        (2, 1): h * _A21,
        (3, 1): h * _A31, (3, 2): h * _A32,
        (4, 1): h * _A41, (4, 2): h * _A42, (4, 3): h * _A43,
        (5, 1): h * _A51, (5, 2): h * _A52, (5, 3): h * _A53, (5, 4): h * _A54,
        (6, 1): h * _A61, (6, 2): h * _A62, (6, 3): h * _A63, (6, 4): h * _A64, (6, 5): h * _A65,
    }
    hB = {j: h * v for j, v in enumerate((_B1, _B2, _B3, _B4, _B5, _B6), start=1)}

    nc = bacc.Bacc()

    y0_d = nc.declare_dram_parameter("y0", [64, 2 * NH], f32, isOutput=False)
    y016_d = nc.declare_dram_parameter("y016", [64, 2 * NH], f16, isOutput=False)
    id64_d = nc.declare_dram_parameter("id64", [64, 64], f16, isOutput=False)
    # L1 stationaries: [variant(yid=0, idy=1)][m-block][half]
    w1cu_d = {
        (v, m, x): nc.declare_dram_parameter(f"w1cu{v}{m}{x}", [128, 128], f16, isOutput=False)
        for v in range(2) for m in range(2) for x in range(2)
    }
    w2t_d = nc.declare_dram_parameter("w2t", [128, 512], f16, isOutput=False)
    w3td_d = nc.declare_dram_parameter("w3td", [128, 256], f16, isOutput=False)
    cb2_d = nc.declare_dram_parameter("cb2", [128, 128], f16, isOutput=False)
    cv_d = nc.declare_dram_parameter("cv", [128, 4], f32, isOutput=False)
    if b3_nonzero:
        cb3_d = nc.declare_dram_parameter("cb3", [128, 64], f16, isOutput=False)
    yout_d = nc.declare_dram_parameter("yout", [64, 2 * NH], f32, isOutput=True)

    with tile.TileContext(nc) as tc:
        with (
            tc.tile_pool(name="const", bufs=1) as cpool,
            tc.tile_pool(name="state", bufs=1) as spool,
            tc.tile_pool(name="act", bufs=2) as apool,
            tc.tile_pool(name="psum", bufs=2, space="PSUM") as ppool,
        ):
            # ---- constants ----
            w1cu = {k: cpool.tile([128, 128], f16, name=f"w1cu{k[0]}{k[1]}{k[2]}") for k in w1cu_d}
            w2t = cpool.tile([128, 512], f16)
            w3td = cpool.tile([128, 256], f16)
            cb2 = cpool.tile([128, 128], f16)
            cv = cpool.tile([128, 4], f32)
            ident = cpool.tile([128, 128], f16)  # [Id64 | Id64] stationary for prefill
            zerot = cpool.tile([128, 128], f16)
            if b3_nonzero:
                cb3 = cpool.tile([128, 64], f16)
                nc.sync.dma_start(cb3[:], cb3_d[:])

            # warmup matmuls: keep the PE activity window busy while DMAs land
            # (reuse the pa1 PSUM tags -- all 8 banks are spoken for)
            nc.gpsimd.memset(zerot[:], 0.0)
            for i in range(N_WARMUP_MM):
                pwarm = ppool.tile([128, 128], f32, tag=f"pa1_{i % 2}", bufs=1, name="pwarm")
                nc.tensor.matmul(pwarm[:], zerot[:], zerot[:], start=True, stop=True)

            for k in w1cu:
                nc.sync.dma_start(w1cu[k][:], w1cu_d[k][:])
            nc.sync.dma_start(w2t[:], w2t_d[:])
            nc.sync.dma_start(w3td[:], w3td_d[:])
            nc.sync.dma_start(cb2[:], cb2_d[:])
            nc.sync.dma_start(cv[:], cv_d[:])
            nc.sync.dma_start(ident[0:64, 0:64], id64_d[:])
            nc.sync.dma_start(ident[64:128, 64:128], id64_d[:])
            nc.gpsimd.memset(ident[0:64, 64:128], 0.0)
            nc.gpsimd.memset(ident[64:128, 0:64], 0.0)

            cv34_1 = cv[:, 0:1]
            cv56_1 = cv[:, 1:2]
            cv56_2 = cv[:, 2:3]
            cv56_3 = cv[:, 3:4]

            # ---- per-half state ----
            # z tiles: variant 0 ([y; I]) for stages 1,2,3,5; variant 1
            # ([I; y]) for stages 4,6
            zvar = {1: 0, 2: 0, 3: 0, 4: 1, 5: 0, 6: 1}
            # merged AB state: cols 0:64 = half A, cols 64:128 = half B
            G = {}
            G["ycur"] = spool.tile([128, 2 * NH], f32, name="ycur")
            G["ynew"] = spool.tile([128, 2 * NH], f32, name="ynew")
            G["P34"] = spool.tile([128, 2 * NH], f32, name="P34")
            G["P56"] = spool.tile([128, 2 * NH], f32, name="P56")
            nc.sync.dma_start(G["ycur"][0:64, :], y0_d[:])
            nc.sync.dma_start(G["ycur"][64:128, :], y0_d[:])
            state = []
            for x in range(2):  # halves A=0 (batch cols 0:64), B=1 (64:128)
                st = {}
                st["z"] = {j: spool.tile([128, NH], f16, name=f"z{j}_{x}") for j in range(1, 7)}
                cols = slice(x * NH, (x + 1) * NH)
                nc.sync.dma_start(st["z"][1][0:64, :], y016_d[:, cols])
                for j in range(1, 7):
                    if zvar[j] == 0:
                        nc.sync.dma_start(st["z"][j][64:128, :], id64_d[:])
                    else:
                        nc.sync.dma_start(st["z"][j][0:64, :], id64_d[:])
                state.append(st)

            def stt_v(out, in0, scal, in1):
                nc.vector.scalar_tensor_tensor(out, in0, scal, in1, op0=MUL, op1=ADD)

            LO = slice(0, 64)
            HI = slice(64, 128)

            skew = {"a_relu2": None, "done": False}

            def emit_stage(x, j, step, last_step):
                st = state[x]
                ycur, ynew, P34, P56 = G["ycur"], G["ynew"], G["P34"], G["P56"]
                cols = slice(x * NH, (x + 1) * NH)

                # pa2 prefill first: no input deps, fills PE idle slots early
                pa2 = ppool.tile([128, 2 * NH], f32, tag=f"pa2_{x}", bufs=1, name=f"pa2_{x}")
                nc.tensor.matmul(pa2[:], ident[:], cb2[:], start=True, stop=False)

                # L1: pre1 = [W1y ; cu1] @ [y; I]  (both m-blocks into one tile)
                v = zvar[j]
                zt = st["z"][j]
                pa1 = ppool.tile([128, 2 * NH], f32, tag=f"pa1_{x}", bufs=1, name=f"pa1_{x}")
                mm_l1 = nc.tensor.matmul(pa1[:, 0:NH], w1cu[(v, 0, x)][:], zt[:], start=True, stop=True)
                if x == 1 and not skew["done"] and skew["a_relu2"] is not None:
                    # startup skew: hold half B ~half a stage behind half A so
                    # the two chains dovetail instead of locking in phase
                    add_dep_helper(mm_l1.ins, skew["a_relu2"].ins, sync=True, reason="AB skew")
                    skew["done"] = True
                nc.tensor.matmul(pa1[:, NH:2 * NH], w1cu[(v, 1, x)][:], zt[:], start=True, stop=True)

                a1 = apool.tile([128, 2 * NH], f16, tag=f"a1_{x}", name=f"a1_{x}")
                nc.scalar.activation(a1[:], pa1[:], Relu)

                # L2: pre2 += W2 @ a1 (K=256 over the two a1 col-blocks)
                nc.tensor.matmul(pa2[:, 0:NH], w2t[:, 0:128], a1[:, 0:NH], start=False, stop=False)
                nc.tensor.matmul(pa2[:, NH:2 * NH], w2t[:, 128:256], a1[:, 0:NH], start=False, stop=False)
                nc.tensor.matmul(pa2[:, 0:NH], w2t[:, 256:384], a1[:, NH:2 * NH], start=False, stop=True)
                nc.tensor.matmul(pa2[:, NH:2 * NH], w2t[:, 384:512], a1[:, NH:2 * NH], start=False, stop=True)

                # relu2 alternates ACT/DVE by stage parity to balance engines
                a2 = apool.tile([128, 2 * NH], f16, tag=f"a2_{x}", name=f"a2_{x}")
                if j % 2 == 1:
                    r2 = nc.scalar.activation(a2[:], pa2[:], Relu)
                else:
                    r2 = nc.vector.tensor_scalar(a2[:], pa2[:], 0.0, 0.0, op0=ADD, op1=MAX)
                if x == 0 and j == 1 and step == 0:
                    skew["a_relu2"] = r2

                # L3: [k; k] into this half's columns of the shared pk tile
                if x == 0:
                    pkfull = ppool.tile([128, 2 * NH], f32, tag="pk", bufs=2, name="pk")
                    st_pk[0] = pkfull
                else:
                    pkfull = st_pk[0]
                pk = pkfull[:, cols]
                if b3_nonzero:
                    nc.tensor.matmul(pk, ident[:], cb3[:], start=True, stop=False)
                    nc.tensor.matmul(pk, w3td[:, 0:128], a2[:, 0:NH], start=False, stop=False)
                else:
                    nc.tensor.matmul(pk, w3td[:, 0:128], a2[:, 0:NH], start=True, stop=False)
                nc.tensor.matmul(pk, w3td[:, 128:256], a2[:, NH:2 * NH], start=False, stop=True)

                # on-chain z-write (DVE, PSUM source), per half
                pkLO = pkfull[LO, cols]
                pkHI = pkfull[HI, cols]
                if j == 1:
                    stt_v(st["z"][2][LO, :], pkLO, hA[(2, 1)], ycur[LO, cols])
                elif j == 2:
                    stt_v(st["z"][3][LO, :], pkLO, hA[(3, 2)], P34[LO, cols])
                elif j == 3:
                    stt_v(st["z"][4][HI, :], pkHI, hA[(4, 3)], P34[HI, cols])
                elif j == 4:
                    stt_v(st["z"][5][LO, :], pkLO, hA[(5, 4)], P56[LO, cols])
                elif j == 5:
                    stt_v(st["z"][6][HI, :], pkHI, hA[(6, 5)], P56[HI, cols])
                else:  # j == 6
                    if not last_step:
                        stt_v(st["z"][1][LO, :], pkLO, hB[6], ynew[LO, cols])

                # merged AB fp32 accumulator updates (DVE, PSUM source),
                # emitted once per stage after half B's L3
                if x == 1:
                    pk2 = pkfull[:]
                    if j == 1:
                        stt_v(P34[:], pk2, cv34_1, ycur[:])
                        stt_v(P56[:], pk2, cv56_1, ycur[:])
                        stt_v(ynew[:], pk2, hB[1], ycur[:])
                    elif j == 2:
                        stt_v(P34[HI, :], pkfull[HI, :], hA[(4, 2)], P34[HI, :])
                        stt_v(P56[:], pk2, cv56_2, P56[:])
                        stt_v(ynew[:], pk2, hB[2], ynew[:])
                    elif j == 3:
                        stt_v(P56[:], pk2, cv56_3, P56[:])
                        stt_v(ynew[:], pk2, hB[3], ynew[:])
                    elif j == 4:
                        stt_v(P56[HI, :], pkfull[HI, :], hA[(6, 4)], P56[HI, :])
                        stt_v(ynew[:], pk2, hB[4], ynew[:])
                    elif j == 5:
                        stt_v(ynew[:], pk2, hB[5], ynew[:])
                    else:  # j == 6: ynew becomes y for the next step
                        stt_v(ynew[:], pk2, hB[6], ynew[:])

            st_pk = {}
            for step in range(n_steps):
                last_step = step == n_steps - 1
                for j in range(1, 7):
                    emit_stage(0, j, step, last_step)
                    emit_stage(1, j, step, last_step)
                # swap y buffers: ynew (fully accumulated) is next step's y
                G["ycur"], G["ynew"] = G["ynew"], G["ycur"]

            nc.sync.dma_start(yout_d[:], G["ycur"][0:64, :])

    nc.compile()
    return nc


def kernel(x0, u, W1, b1, W2, b2, W3, b3, t0, t1):
    from concourse.bass_utils import run_bass_kernel_spmd

    x0 = np.asarray(x0, dtype=np.float32)
    u = np.asarray(u, dtype=np.float32)
    W1 = np.asarray(W1, dtype=np.float32)
    W2 = np.asarray(W2, dtype=np.float32)
    W3 = np.asarray(W3, dtype=np.float32)
    b1 = np.asarray(b1, dtype=np.float32)
    b2 = np.asarray(b2, dtype=np.float32)
    b3 = np.asarray(b3, dtype=np.float32)

    Bt, D = x0.shape
    n = Bt // N_CORES
    h = DT0 * SECOND
    n_steps = int(round((float(np.asarray(t1)) - float(np.asarray(t0))) / h))
    b3_nonzero = bool(np.any(b3 != 0))

    nc = _build_program(n_steps, b3_nonzero)

    f16 = np.float16
    W1y = W1[:, 0:64]   # [256, 64]
    W1u = W1[:, 64:128]

    w2T = W2.T.astype(f16)  # [256, 256]
    w2t = np.ascontiguousarray(
        np.concatenate([w2T[0:128, 0:128], w2T[0:128, 128:256], w2T[128:256, 0:128], w2T[128:256, 128:256]], axis=1)
    )
    w3T = W3.T.astype(f16)  # [256, 64]
    w3td = np.ascontiguousarray(
        np.concatenate([w3T[0:128], w3T[0:128], w3T[128:256], w3T[128:256]], axis=1)
    )

    cb2 = np.zeros((128, 128), np.float32)
    cb2[:, 0:64] = b2[0:128, None]
    cb2[:, 64:128] = b2[128:256, None]
    cb2 = cb2.astype(f16)

    cvm = np.zeros((128, 4), np.float32)
    cvm[0:64, 0] = h * _A31
    cvm[64:128, 0] = h * _A41
    cvm[0:64, 1] = h * _A51
    cvm[64:128, 1] = h * _A61
    cvm[0:64, 2] = h * _A52
    cvm[64:128, 2] = h * _A62
    cvm[0:64, 3] = h * _A53
    cvm[64:128, 3] = h * _A63

    id64 = np.eye(64, dtype=f16)

    in_maps = []
    for c in range(N_CORES):
        sl = slice(c * n, (c + 1) * n)
        x0c = x0[sl]  # [128, 64]
        uc = u[sl]    # [128, 64]
        im = {
            "y0": np.ascontiguousarray(x0c.T),
            "y016": np.ascontiguousarray(x0c.T.astype(f16)),
            "id64": id64,
            "w2t": w2t,
            "w3td": w3td,
            "cb2": cb2,
            "cv": cvm,
        }
        if b3_nonzero:
            cb3 = np.zeros((128, 64), np.float32)
            cb3[0:64] = b3[:, None]
            cb3[64:128] = b3[:, None]
            im["cb3"] = cb3.astype(f16)
        for x in range(2):
            ux = uc[x * NH:(x + 1) * NH]  # [64, 64] batch-major
            cu1 = W1u @ ux.T + b1[:, None]  # [256, 64]
            for m in range(2):
                w1yT = W1y.T[:, m * 128:(m + 1) * 128]          # [64, 128]
                cu1T = cu1[m * 128:(m + 1) * 128, :].T          # [64, 128]
                im[f"w1cu0{m}{x}"] = np.ascontiguousarray(
                    np.concatenate([w1yT, cu1T], axis=0).astype(f16))
                im[f"w1cu1{m}{x}"] = np.ascontiguousarray(
                    np.concatenate([cu1T, w1yT], axis=0).astype(f16))
        in_maps.append(im)

    res = run_bass_kernel_spmd(nc, in_maps, list(range(N_CORES)))
    globals()["LAST_RESULT"] = res

    out = np.empty((Bt, D), np.float32)
    for c in range(N_CORES):
        out[c * n:(c + 1) * n, :] = res.results[c]["yout"].T
    return out
